# revision 3
# baseline (speedup 1.0000x reference)
"""FreeConv2D (locally-connected conv2d + bias) Trainium2 Bass kernel.

out[b,oh,ow,u] = sum_{i,j,c} w[oh,ow,u,i,j,c] * x[b, oh*2+i, ow*2+j, c] + bias[oh,ow,u]

Shapes: x [64,64,64,64], w [30,30,64,5,5,64], b [30,30,64] -> out [64,30,30,64].

Strategy (8 NeuronCores):
  - Shard output rows OH over cores: 4 rows/core (padded 30->32; last 2 dummy).
  - Host pre-packs (numpy, not counted in HW time):
      * x    -> per-core fp16 tile [128, 11*32*64]: partition p = dj*64+c for
               column pair (2*mp, 2*mp+1), free = (r, mp, b).
      * w    -> per-core fp16 stream [128, TOT]: sequence of matmul rhs blocks
               in execution order (column-pair taps j in {0,1} / {2,3} as
               K=128 blocks; j=4 taps as K=64 blocks zero-padded to 128 rows).
      * bias -> per-core fp32 [64, 30*4*64] replicated over batch partitions.
  - Device: 32-phase sweep over column pairs mp. Phase mp:
      * DMA the phase's w blocks (~1 MB).
      * psum phase tile pt[mp] [64, 512] = accum slots (oh_l, role) where
        role 0 = j01-half of loc (oh, mp), role 1 = j23-half of loc (oh, mp-1).
      * matmuls: lhsT = resident x tile [128, 64(b)] (stationary),
        rhs = w blocks [128, N<=512] (moving), accumulate with start=False
        (tiles pre-zeroed by DVE memset; psum has_written semantics make this
        correct whether the first PE write accumulates or overwrites).
      * j=4 taps (K=64) of loc (oh, mp-2) also land in pt[mp-2] role-0 slots.
      * drain loc (.., ow=mp-2): out = pt[ow].role0 + bias + pt[ow+1].role1
        via two DVE tensor_adds into an SBUF staging buffer.
  - One final DMA of staging -> DRAM out [64, 30(ow), 4(oh_l), 64] per core;
    host gathers/transposes/trims to [64, 30, 30, 64].
"""

import os
import sys

import numpy as np

_TRN_REPO = "/opt/trn_rl_repo"
if _TRN_REPO not in sys.path:
    sys.path.insert(0, _TRN_REPO)

# ---------------- problem constants (hardcoded) ----------------
B, H, W, C = 64, 64, 64, 64
U, K, S = 64, 5, 2
OH = OW = 30
NCORES = 8
NO = 4                      # oh rows per core (padded: 8*4 = 32 >= 30)
OHP = NCORES * NO           # 32
NR = 2 * (NO - 1) + K       # 11 input rows per core
NMP = 32                    # column-pair tiles mp=0..31; also phase count
HP = 2 * (OHP - 1) + K      # 67 padded input rows overall


def _oh_span(r):
    """Valid local oh range for local input row r: i = r - 2*oh in [0, K-1]."""
    lo = max(0, -(-(r - (K - 1)) // 2))   # ceil((r-4)/2)
    hi = min(NO - 1, r // 2)
    return lo, hi


def build_schedule():
    """Per-phase block lists. Block cols are offsets into the packed w stream.

    Returns (phases, totcols, wmax) where phases[mp] is a list of dicts:
      kind 'main': K=128 block, taps j=(0,1) for ow=mp [role 0] and/or
                   j=(2,3) for ow=mp-1 [role 1]; ncols = noh*nroles*64.
      kind 'j4':   K=64 block (rows 64:128 zero), tap j=4 for ow=mp-2;
                   ncols = noh*64.
    """
    phases = []
    col = 0
    wmax = 0
    for mp in range(NMP):
        blocks = []
        for r in range(NR):
            if mp <= OW:  # main blocks exist for mp=0..30
                roles = []
                if mp <= OW - 1:
                    roles.append(0)          # a1: loc (oh, mp), j in {0,1}
                if mp >= 1:
                    roles.append(1)          # a2: loc (oh, mp-1), j in {2,3}
                lo, hi = _oh_span(r)
                if roles and lo <= hi:
                    ncols = (hi - lo + 1) * len(roles) * U
                    blocks.append(dict(kind="main", r=r, mp=mp, col0=col,
                                       ncols=ncols, oh0=lo, noh=hi - lo + 1,
                                       roles=tuple(roles)))
                    col += ncols
        for r in range(NR):
            ow4 = mp - 2
            if 0 <= ow4 <= OW - 1:
                lo, hi = _oh_span(r)
                if lo <= hi:
                    ncols = (hi - lo + 1) * U
                    blocks.append(dict(kind="j4", r=r, mp=mp, col0=col,
                                       ncols=ncols, oh0=lo, noh=hi - lo + 1))
                    col += ncols
        pc = sum(bl["ncols"] for bl in blocks)
        wmax = max(wmax, pc)
        phases.append(blocks)
    return phases, col, wmax


def pack_inputs(x, w, b):
    """Build the per-core input arrays. Returns list of dicts for in_maps."""
    x = np.ascontiguousarray(np.asarray(x, dtype=np.float32))
    w = np.asarray(w, dtype=np.float32)
    b = np.asarray(b, dtype=np.float32)

    phases, totcols, _ = build_schedule()

    # x: pad rows to HP, transpose to [h, w, c, b] fp16
    xT = np.zeros((HP, W, C, B), dtype=np.float16)
    xT[:H] = x.transpose(1, 2, 3, 0).astype(np.float16)

    # w: [OH,OW,U,K,K,C] -> wt [OHP, OW, K(i), K(j), C, U] fp16, padded oh rows
    wt = np.zeros((OHP, OW, K, K, C, U), dtype=np.float16)
    wt[:OH] = w.transpose(0, 1, 3, 4, 5, 2).astype(np.float16)

    bias_pad = np.zeros((OHP, OW, U), dtype=np.float32)
    bias_pad[:OH] = b

    in_maps = []
    for core in range(NCORES):
        oh0 = core * NO
        r0 = 2 * oh0
        # x tile: [128, NR*NMP*B]; p = dj*64+c ; free = (r, mp, b)
        xc = xT[r0:r0 + NR]                                  # [NR, W, C, B]
        xc = xc.reshape(NR, NMP, 2, C, B)                    # [r, mp, dj, c, b]
        xtile = np.ascontiguousarray(
            xc.transpose(2, 3, 0, 1, 4).reshape(128, NR * NMP * B))

        # w stream
        ws = np.zeros((128, totcols), dtype=np.float16)
        for mp, blocks in enumerate(phases):
            for bl in blocks:
                r = bl["r"]
                lo, noh = bl["oh0"], bl["noh"]
                ohs = np.arange(lo, lo + noh)
                i_s = r - 2 * ohs
                ohs_g = oh0 + ohs
                c0 = bl["col0"]
                if bl["kind"] == "main":
                    for k, role in enumerate(bl["roles"]):
                        ow = mp if role == 0 else mp - 1
                        j0 = 0 if role == 0 else 2
                        # [noh, 2(dj), C, U]
                        src = wt[ohs_g, ow, i_s, j0:j0 + 2]
                        # -> [128=(dj,c), noh, U] -> per-(oh,role) col chunks
                        blk = src.transpose(1, 2, 0, 3).reshape(128, noh, U)
                        nroles = len(bl["roles"])
                        for t in range(noh):
                            cc = c0 + (t * nroles + k) * U
                            ws[:, cc:cc + U] = blk[:, t, :]
                else:
                    ow4 = mp - 2
                    src = wt[ohs_g, ow4, i_s, 4]             # [noh, C, U]
                    blk = src.transpose(1, 0, 2).reshape(C, noh * U)
                    ws[0:C, c0:c0 + bl["ncols"]] = blk

        # bias: [64, OW*NO*U] fp32, (ow, oh_l, u) order, replicated over b
        bc = bias_pad[oh0:oh0 + NO].transpose(1, 0, 2).reshape(1, OW * NO * U)
        bias_rep = np.ascontiguousarray(
            np.broadcast_to(bc, (64, OW * NO * U)).astype(np.float32))

        in_maps.append({"xt": xtile, "wstream": ws, "bias_rep": bias_rep})
    return in_maps


def emulate_core(inp):
    """Numpy emulation of the device program for one core (validation)."""
    phases, totcols, _ = build_schedule()
    xt = inp["xt"].astype(np.float32)
    ws = inp["wstream"].astype(np.float32)
    bias = inp["bias_rep"]
    pts = {}
    stag = np.zeros((64, OW, NO, U), dtype=np.float32)
    for mp, blocks in enumerate(phases):
        if mp <= OW:
            pts[mp] = np.zeros((64, NO, 2, U), dtype=np.float32)
        for bl in blocks:
            r = bl["r"]
            lo, noh = bl["oh0"], bl["noh"]
            lhsT = xt[:, (r * NMP + mp) * B:(r * NMP + mp) * B + B]  # [128, 64]
            rhs = ws[:, bl["col0"]:bl["col0"] + bl["ncols"]]
            if bl["kind"] == "main":
                res = lhsT.T @ rhs                       # [64, noh*nroles*64]
                res = res.reshape(64, noh, len(bl["roles"]), U)
                for k, role in enumerate(bl["roles"]):
                    pts[mp][:, lo:lo + noh, role, :] += res[:, :, k, :]
            else:
                res = lhsT[0:64].T @ rhs[0:64]           # [64, noh*64]
                pts[mp - 2][:, lo:lo + noh, 0, :] += res.reshape(64, noh, U)
        ow = mp - 2
        if 0 <= ow <= OW - 1:
            a1 = pts[ow][:, :, 0, :]
            a2 = pts[ow + 1][:, :, 1, :]
            bv = bias[:, ow * NO * U:(ow + 1) * NO * U].reshape(64, NO, U)
            stag[:, ow] = a1 + bv + a2
    return stag  # [64, ow, oh_l, u]


# ---------------- device kernel ----------------

def build_nc(dma_engine="sync"):
    import concourse.bass as bass  # noqa: F401
    import concourse.mybir as mybir
    import concourse.tile as tile
    from concourse import bacc

    phases, totcols, wmax = build_schedule()
    dt = mybir.dt

    nc = bacc.Bacc("TRN2", target_bir_lowering=False, debug=False,
                   num_devices=NCORES)
    xt_d = nc.dram_tensor("xt", [128, NR * NMP * B], dt.float16,
                          kind="ExternalInput").ap()
    ws_d = nc.dram_tensor("wstream", [128, totcols], dt.float16,
                          kind="ExternalInput").ap()
    bias_d = nc.dram_tensor("bias_rep", [64, OW * NO * U], dt.float32,
                            kind="ExternalInput").ap()
    out_d = nc.dram_tensor("out", [B, OW, NO, U], dt.float32,
                           kind="ExternalOutput").ap()

    with tile.TileContext(nc) as tc:
        with tc.tile_pool(name="xpool", bufs=1) as xpool, \
             tc.tile_pool(name="bpool", bufs=1) as bpool, \
             tc.tile_pool(name="stpool", bufs=1) as stpool, \
             tc.tile_pool(name="wpool", bufs=3) as wpool, \
             tc.tile_pool(name="tmppool", bufs=4) as tmppool, \
             tc.tile_pool(name="pspool", bufs=5, space="PSUM") as pspool:

            dma = getattr(nc, dma_engine)

            xsb = xpool.tile([128, NR * NMP * B], dt.float16)
            dma.dma_start(xsb[:, :], xt_d[:, :])
            bsb = bpool.tile([64, OW * NO * U], dt.float32)
            dma.dma_start(bsb[:, :], bias_d[:, :])
            stag = stpool.tile([64, OW * NO * U], dt.float32)

            pts = {}
            for mp, blocks in enumerate(phases):
                pc0 = blocks[0]["col0"]
                wcols = sum(bl["ncols"] for bl in blocks)
                wsb = wpool.tile([128, wmax], dt.float16, tag="wstream")
                dma.dma_start(wsb[:, :wcols], ws_d[:, pc0:pc0 + wcols])

                if mp <= OW:
                    pt = pspool.tile([64, NO * 2 * U], dt.float32)
                    pts[mp] = pt
                    nc.vector.memset(pt[:, :], 0.0)

                for bl in blocks:
                    r = bl["r"]
                    lo, noh = bl["oh0"], bl["noh"]
                    xoff = (r * NMP + mp) * B
                    loc0 = bl["col0"] - pc0
                    if bl["kind"] == "main":
                        ptv = pts[mp][:, :].rearrange(
                            "p (o q) -> p o q", o=NO, q=2 * U)
                        lhsT = xsb[:, xoff:xoff + B]
                        rhs = wsb[:, loc0:loc0 + bl["ncols"]]
                        if len(bl["roles"]) == 2:
                            outap = ptv[:, lo:lo + noh, :]
                        elif bl["roles"][0] == 0:
                            outap = ptv[:, lo:lo + noh, 0:U]
                        else:
                            outap = ptv[:, lo:lo + noh, U:2 * U]
                        nc.tensor.matmul(outap, lhsT, rhs, start=False,
                                         stop=False, skip_group_check=True)
                    else:
                        ptv = pts[mp - 2][:, :].rearrange(
                            "p (o q) -> p o q", o=NO, q=2 * U)
                        lhsT = xsb[0:64, xoff:xoff + B]
                        rhs = wsb[0:64, loc0:loc0 + bl["ncols"]]
                        outap = ptv[:, lo:lo + noh, 0:U]
                        nc.tensor.matmul(outap, lhsT, rhs, start=False,
                                         stop=False, skip_group_check=True)

                ow = mp - 2
                if 0 <= ow <= OW - 1:
                    a1 = pts[ow][:, :].rearrange(
                        "p (o q) -> p o q", o=NO, q=2 * U)[:, :, 0:U]
                    a2 = pts[ow + 1][:, :].rearrange(
                        "p (o q) -> p o q", o=NO, q=2 * U)[:, :, U:2 * U]
                    bv = bsb[:, ow * NO * U:(ow + 1) * NO * U].rearrange(
                        "p (o u) -> p o u", u=U)
                    tmp = tmppool.tile([64, NO * U], dt.float32)
                    tmpv = tmp[:, :].rearrange("p (o u) -> p o u", u=U)
                    nc.vector.tensor_add(tmpv, a1, bv)
                    stv = stag[:, ow * NO * U:(ow + 1) * NO * U].rearrange(
                        "p (o u) -> p o u", u=U)
                    nc.vector.tensor_add(stv, tmpv, a2)
                    del pts[ow]

            dma.dma_start(out_d.rearrange("b w o u -> b (w o u)"), stag[:, :])

    nc.compile()
    return nc


def _exec(nc, in_maps, repeats=1):
    """Execute the prebuilt Bass module on the 8 cores via PJRT/axon.

    Mirrors bass2jax.run_bass_via_pjrt's multi-core branch, but keeps the
    jitted executable + device-staged inputs so the kernel can be re-run for
    timing. Returns (per_core_results, wall_times_s).
    """
    import time

    import jax
    import numpy as _np
    from jax.sharding import Mesh, NamedSharding, PartitionSpec

    try:
        from jax.experimental.shard_map import shard_map
    except ImportError:
        from jax.shard_map import shard_map

    import concourse.mybir as mybir
    from concourse import bass2jax

    bass2jax.install_neuronx_cc_hook()

    partition_name = (nc.partition_id_tensor.name
                      if nc.partition_id_tensor else None)
    in_names, out_names, out_avals, zero_outs = [], [], [], []
    for alloc in nc.m.functions[0].allocations:
        if not isinstance(alloc, mybir.MemoryLocationSet):
            continue
        name = alloc.memorylocations[0].name
        if alloc.kind == "ExternalInput":
            if name != partition_name:
                in_names.append(name)
        elif alloc.kind == "ExternalOutput":
            out_names.append(name)
            shape = tuple(alloc.tensor_shape)
            dtype = mybir.dt.np(alloc.dtype)
            out_avals.append(jax.core.ShapedArray(shape, dtype))
            zero_outs.append(_np.zeros(shape, dtype))
    n_params = len(in_names)
    all_names = in_names + out_names
    if partition_name is not None:
        all_names = all_names + [partition_name]

    def _body(*args):
        operands = list(args)
        if partition_name is not None:
            operands.append(bass2jax.partition_id_tensor())
        outs = bass2jax._bass_exec_p.bind(
            *operands,
            out_avals=tuple(out_avals),
            in_names=tuple(all_names),
            out_names=tuple(out_names),
            lowering_input_output_aliases=(),
            sim_require_finite=True,
            sim_require_nnan=True,
            nc=nc,
        )
        return tuple(outs)

    n_cores = len(in_maps)
    devices = jax.devices()[:n_cores]
    mesh = Mesh(_np.asarray(devices), ("core",))
    spec = PartitionSpec("core")
    sharded = jax.jit(
        shard_map(_body, mesh=mesh, in_specs=(spec,) * (n_params + len(out_names)),
                  out_specs=(spec,) * len(out_names), check_rep=False),
        keep_unused=True,
    )
    sharding = NamedSharding(mesh, spec)
    staged = [
        jax.device_put(
            _np.concatenate([_np.asarray(m[name]) for m in in_maps], axis=0),
            sharding)
        for name in in_names
    ] + [
        jax.device_put(
            _np.zeros((n_cores * z.shape[0], *z.shape[1:]), z.dtype), sharding)
        for z in zero_outs
    ]

    times = []
    out_arrs = None
    for _ in range(max(1, repeats)):
        t0 = time.perf_counter()
        out_arrs = jax.block_until_ready(sharded(*staged))
        times.append(time.perf_counter() - t0)

    results = [
        {
            name: _np.asarray(out_arrs[i]).reshape(n_cores, *out_avals[i].shape)[c]
            for i, name in enumerate(out_names)
        }
        for c in range(n_cores)
    ]
    return results, times


def _run(inputs, repeats=1):
    """Run on hardware. Returns (full_output, wall_times_s)."""
    in_maps = pack_inputs(inputs["x"], inputs["w"], inputs["b"])
    nc = build_nc()
    results, times = _exec(nc, in_maps, repeats=repeats)
    out = np.empty((B, OHP, OW, U), dtype=np.float32)
    for c in range(NCORES):
        # per-core out [B, OW, NO, U] -> [B, NO, OW, U]
        out[:, c * NO:(c + 1) * NO] = results[c]["out"].transpose(0, 2, 1, 3)
    return out[:, :OH], times


def kernel(x, w, b):
    out, _ = _run({"x": x, "w": w, "b": b})
    return out


# revision 5
# speedup vs baseline: 4076.4811x; 4076.4811x over previous
"""FreeConv2D (locally-connected conv2d + bias) Trainium2 Bass kernel.

out[b,oh,ow,u] = sum_{i,j,c} w[oh,ow,u,i,j,c] * x[b, oh*2+i, ow*2+j, c] + bias[oh,ow,u]

Shapes: x [64,64,64,64], w [30,30,64,5,5,64], b [30,30,64] -> out [64,30,30,64].

Strategy (8 NeuronCores):
  - Shard output rows OH over cores: 4 rows/core (padded 30->32; last 2 dummy).
  - Host pre-packs (numpy, not counted in HW time):
      * x    -> per-core fp16 tile [128, 11*32*64]: partition p = dj*64+c for
               column pair (2*mp, 2*mp+1), free = (r, mp, b).
      * w    -> per-core fp16 stream [128, TOT]: sequence of matmul rhs blocks
               in execution order (column-pair taps j in {0,1} / {2,3} as
               K=128 blocks; j=4 taps as K=64 blocks zero-padded to 128 rows).
      * bias -> per-core fp32 [64, 30*4*64] replicated over batch partitions.
  - Device: 32-phase sweep over column pairs mp. Phase mp:
      * DMA the phase's w blocks (~1 MB).
      * psum phase tile pt[mp] [64, 512] = accum slots (oh_l, role) where
        role 0 = j01-half of loc (oh, mp), role 1 = j23-half of loc (oh, mp-1).
      * matmuls: lhsT = resident x tile [128, 64(b)] (stationary),
        rhs = w blocks [128, N<=512] (moving), accumulate with start=False
        (tiles pre-zeroed by DVE memset; psum has_written semantics make this
        correct whether the first PE write accumulates or overwrites).
      * j=4 taps (K=64) of loc (oh, mp-2) also land in pt[mp-2] role-0 slots.
      * drain loc (.., ow=mp-2): out = pt[ow].role0 + bias + pt[ow+1].role1
        via two DVE tensor_adds into an SBUF staging buffer.
  - One final DMA of staging -> DRAM out [64, 30(ow), 4(oh_l), 64] per core;
    host gathers/transposes/trims to [64, 30, 30, 64].
"""

import os
import sys

import numpy as np

_TRN_REPO = "/opt/trn_rl_repo"
if _TRN_REPO not in sys.path:
    sys.path.insert(0, _TRN_REPO)

# ---------------- problem constants (hardcoded) ----------------
B, H, W, C = 64, 64, 64, 64
U, K, S = 64, 5, 2
OH = OW = 30
NCORES = 8
NO = 4                      # oh rows per core (padded: 8*4 = 32 >= 30)
OHP = NCORES * NO           # 32
NR = 2 * (NO - 1) + K       # 11 input rows per core
NMP = 32                    # column-pair tiles mp=0..31; also phase count
HP = 2 * (OHP - 1) + K      # 67 padded input rows overall


def _oh_span(r):
    """Valid local oh range for local input row r: i = r - 2*oh in [0, K-1]."""
    lo = max(0, -(-(r - (K - 1)) // 2))   # ceil((r-4)/2)
    hi = min(NO - 1, r // 2)
    return lo, hi


def build_schedule():
    """Per-phase block lists. Block cols are offsets into the packed w stream.

    Returns (phases, totcols, wmax) where phases[mp] is a list of dicts:
      kind 'main': K=128 block, taps j=(0,1) for ow=mp [role 0] and/or
                   j=(2,3) for ow=mp-1 [role 1]; ncols = noh*nroles*64.
      kind 'j4':   K=64 block (rows 64:128 zero), tap j=4 for ow=mp-2;
                   ncols = noh*64.
    """
    phases = []
    col = 0
    wmax = 0
    for mp in range(NMP):
        blocks = []
        for r in range(NR):
            if mp <= OW:  # main blocks exist for mp=0..30
                roles = []
                if mp <= OW - 1:
                    roles.append(0)          # a1: loc (oh, mp), j in {0,1}
                if mp >= 1:
                    roles.append(1)          # a2: loc (oh, mp-1), j in {2,3}
                lo, hi = _oh_span(r)
                if roles and lo <= hi:
                    ncols = (hi - lo + 1) * len(roles) * U
                    blocks.append(dict(kind="main", r=r, mp=mp, col0=col,
                                       ncols=ncols, oh0=lo, noh=hi - lo + 1,
                                       roles=tuple(roles)))
                    col += ncols
        for r in range(NR):
            ow4 = mp - 2
            if 0 <= ow4 <= OW - 1:
                lo, hi = _oh_span(r)
                if lo <= hi:
                    ncols = (hi - lo + 1) * U
                    blocks.append(dict(kind="j4", r=r, mp=mp, col0=col,
                                       ncols=ncols, oh0=lo, noh=hi - lo + 1))
                    col += ncols
        pc = sum(bl["ncols"] for bl in blocks)
        wmax = max(wmax, pc)
        phases.append(blocks)
    return phases, col, wmax


def pack_inputs(x, w, b):
    """Build the per-core input arrays. Returns list of dicts for in_maps."""
    x = np.ascontiguousarray(np.asarray(x, dtype=np.float32))
    w = np.asarray(w, dtype=np.float32)
    b = np.asarray(b, dtype=np.float32)

    phases, totcols, _ = build_schedule()

    # x: pad rows to HP, transpose to [h, w, c, b] fp16
    xT = np.zeros((HP, W, C, B), dtype=np.float16)
    xT[:H] = x.transpose(1, 2, 3, 0).astype(np.float16)

    # w: [OH,OW,U,K,K,C] -> wt [OHP, OW, K(i), K(j), C, U] fp16, padded oh rows
    wt = np.zeros((OHP, OW, K, K, C, U), dtype=np.float16)
    wt[:OH] = w.transpose(0, 1, 3, 4, 5, 2).astype(np.float16)

    bias_pad = np.zeros((OHP, OW, U), dtype=np.float32)
    bias_pad[:OH] = b

    in_maps = []
    for core in range(NCORES):
        oh0 = core * NO
        r0 = 2 * oh0
        # x tile: [128, NR*NMP*B]; p = dj*64+c ; free = (r, mp, b)
        xc = xT[r0:r0 + NR]                                  # [NR, W, C, B]
        xc = xc.reshape(NR, NMP, 2, C, B)                    # [r, mp, dj, c, b]
        xtile = np.ascontiguousarray(
            xc.transpose(2, 3, 0, 1, 4).reshape(128, NR * NMP * B))

        # w stream
        ws = np.zeros((128, totcols), dtype=np.float16)
        for mp, blocks in enumerate(phases):
            for bl in blocks:
                r = bl["r"]
                lo, noh = bl["oh0"], bl["noh"]
                ohs = np.arange(lo, lo + noh)
                i_s = r - 2 * ohs
                ohs_g = oh0 + ohs
                c0 = bl["col0"]
                if bl["kind"] == "main":
                    for k, role in enumerate(bl["roles"]):
                        ow = mp if role == 0 else mp - 1
                        j0 = 0 if role == 0 else 2
                        # [noh, 2(dj), C, U]
                        src = wt[ohs_g, ow, i_s, j0:j0 + 2]
                        # -> [128=(dj,c), noh, U] -> per-(oh,role) col chunks
                        blk = src.transpose(1, 2, 0, 3).reshape(128, noh, U)
                        nroles = len(bl["roles"])
                        for t in range(noh):
                            cc = c0 + (t * nroles + k) * U
                            ws[:, cc:cc + U] = blk[:, t, :]
                else:
                    ow4 = mp - 2
                    src = wt[ohs_g, ow4, i_s, 4]             # [noh, C, U]
                    blk = src.transpose(1, 0, 2).reshape(C, noh * U)
                    ws[0:C, c0:c0 + bl["ncols"]] = blk

        # bias: [64, OW*NO*U] fp32, (ow, oh_l, u) order, replicated over b
        bc = bias_pad[oh0:oh0 + NO].transpose(1, 0, 2).reshape(1, OW * NO * U)
        bias_rep = np.ascontiguousarray(
            np.broadcast_to(bc, (64, OW * NO * U)).astype(np.float32))

        in_maps.append({"xt": xtile, "wstream": ws, "bias_rep": bias_rep})
    return in_maps


def emulate_core(inp):
    """Numpy emulation of the device program for one core (validation)."""
    phases, totcols, _ = build_schedule()
    xt = inp["xt"].astype(np.float32)
    ws = inp["wstream"].astype(np.float32)
    bias = inp["bias_rep"]
    pts = {}
    stag = np.zeros((64, OW, NO, U), dtype=np.float32)
    for mp, blocks in enumerate(phases):
        if mp <= OW:
            pts[mp] = np.zeros((64, NO, 2, U), dtype=np.float32)
        for bl in blocks:
            r = bl["r"]
            lo, noh = bl["oh0"], bl["noh"]
            lhsT = xt[:, (r * NMP + mp) * B:(r * NMP + mp) * B + B]  # [128, 64]
            rhs = ws[:, bl["col0"]:bl["col0"] + bl["ncols"]]
            if bl["kind"] == "main":
                res = lhsT.T @ rhs                       # [64, noh*nroles*64]
                res = res.reshape(64, noh, len(bl["roles"]), U)
                for k, role in enumerate(bl["roles"]):
                    pts[mp][:, lo:lo + noh, role, :] += res[:, :, k, :]
            else:
                res = lhsT[0:64].T @ rhs[0:64]           # [64, noh*64]
                pts[mp - 2][:, lo:lo + noh, 0, :] += res.reshape(64, noh, U)
        ow = mp - 2
        if 0 <= ow <= OW - 1:
            a1 = pts[ow][:, :, 0, :]
            a2 = pts[ow + 1][:, :, 1, :]
            bv = bias[:, ow * NO * U:(ow + 1) * NO * U].reshape(64, NO, U)
            stag[:, ow] = a1 + bv + a2
    return stag  # [64, ow, oh_l, u]


# ---------------- device kernel ----------------

def build_nc(dma_engine="sync"):
    import concourse.bass as bass  # noqa: F401
    import concourse.mybir as mybir
    import concourse.tile as tile
    from concourse import bacc

    phases, totcols, wmax = build_schedule()
    dt = mybir.dt

    nc = bacc.Bacc("TRN2", target_bir_lowering=False, debug=False,
                   num_devices=NCORES)
    xt_d = nc.dram_tensor("xt", [128, NR * NMP * B], dt.float16,
                          kind="ExternalInput").ap()
    ws_d = nc.dram_tensor("wstream", [128, totcols], dt.float16,
                          kind="ExternalInput").ap()
    bias_d = nc.dram_tensor("bias_rep", [64, OW * NO * U], dt.float32,
                            kind="ExternalInput").ap()
    out_d = nc.dram_tensor("out", [B, OW, NO, U], dt.float32,
                           kind="ExternalOutput").ap()

    with tile.TileContext(nc) as tc:
        with tc.tile_pool(name="xpool", bufs=1) as xpool, \
             tc.tile_pool(name="bpool", bufs=1) as bpool, \
             tc.tile_pool(name="stpool", bufs=1) as stpool, \
             tc.tile_pool(name="wpool", bufs=3) as wpool, \
             tc.tile_pool(name="tmppool", bufs=4) as tmppool, \
             tc.tile_pool(name="pspool", bufs=5, space="PSUM") as pspool:

            dma = getattr(nc, dma_engine)

            xsb = xpool.tile([128, NR * NMP * B], dt.float16)
            dma.dma_start(xsb[:, :], xt_d[:, :])
            bsb = bpool.tile([64, OW * NO * U], dt.float32)
            dma.dma_start(bsb[:, :], bias_d[:, :])
            stag = stpool.tile([64, OW * NO * U], dt.float32)

            pts = {}
            for mp, blocks in enumerate(phases):
                pc0 = blocks[0]["col0"]
                wcols = sum(bl["ncols"] for bl in blocks)
                wsb = wpool.tile([128, wmax], dt.float16, tag="wstream")
                dma.dma_start(wsb[:, :wcols], ws_d[:, pc0:pc0 + wcols])

                if mp <= OW:
                    pt = pspool.tile([64, NO * 2 * U], dt.float32)
                    pts[mp] = pt
                    nc.vector.memset(pt[:, :], 0.0)

                for bl in blocks:
                    r = bl["r"]
                    lo, noh = bl["oh0"], bl["noh"]
                    xoff = (r * NMP + mp) * B
                    loc0 = bl["col0"] - pc0
                    if bl["kind"] == "main":
                        ptv = pts[mp][:, :].rearrange(
                            "p (o q) -> p o q", o=NO, q=2 * U)
                        lhsT = xsb[:, xoff:xoff + B]
                        rhs = wsb[:, loc0:loc0 + bl["ncols"]]
                        if len(bl["roles"]) == 2:
                            outap = ptv[:, lo:lo + noh, :]
                        elif bl["roles"][0] == 0:
                            outap = ptv[:, lo:lo + noh, 0:U]
                        else:
                            outap = ptv[:, lo:lo + noh, U:2 * U]
                        nc.tensor.matmul(outap, lhsT, rhs, start=False,
                                         stop=False, skip_group_check=True)
                    else:
                        ptv = pts[mp - 2][:, :].rearrange(
                            "p (o q) -> p o q", o=NO, q=2 * U)
                        lhsT = xsb[0:64, xoff:xoff + B]
                        rhs = wsb[0:64, loc0:loc0 + bl["ncols"]]
                        outap = ptv[:, lo:lo + noh, 0:U]
                        nc.tensor.matmul(outap, lhsT, rhs, start=False,
                                         stop=False, skip_group_check=True)

                ow = mp - 2
                if 0 <= ow <= OW - 1:
                    a1 = pts[ow][:, :].rearrange(
                        "p (o q) -> p o q", o=NO, q=2 * U)[:, :, 0:U]
                    a2 = pts[ow + 1][:, :].rearrange(
                        "p (o q) -> p o q", o=NO, q=2 * U)[:, :, U:2 * U]
                    bv = bsb[:, ow * NO * U:(ow + 1) * NO * U].rearrange(
                        "p (o u) -> p o u", u=U)
                    tmp = tmppool.tile([64, NO * U], dt.float32)
                    tmpv = tmp[:, :].rearrange("p (o u) -> p o u", u=U)
                    nc.vector.tensor_add(tmpv, a1, bv)
                    stv = stag[:, ow * NO * U:(ow + 1) * NO * U].rearrange(
                        "p (o u) -> p o u", u=U)
                    nc.vector.tensor_add(stv, tmpv, a2)
                    del pts[ow]

            dma.dma_start(out_d.rearrange("b w o u -> b (w o u)"), stag[:, :])

    nc.compile()
    return nc


def _exec(nc, in_maps, repeats=1, chain=1):
    """Execute the prebuilt Bass module on the 8 cores via PJRT/axon.

    Mirrors bass2jax.run_bass_via_pjrt's multi-core branch, but keeps the
    jitted executable + device-staged inputs so the kernel can be re-run for
    timing. `chain` repeats the kernel execution inside one program (for
    amortized on-device timing). Returns (per_core_results, wall_times_s).
    """
    import time

    import jax
    import numpy as _np
    from jax.sharding import Mesh, NamedSharding, PartitionSpec

    try:
        from jax.experimental.shard_map import shard_map
    except ImportError:
        from jax.shard_map import shard_map

    import concourse.mybir as mybir
    from concourse import bass2jax

    bass2jax.install_neuronx_cc_hook()

    partition_name = (nc.partition_id_tensor.name
                      if nc.partition_id_tensor else None)
    in_names, out_names, out_avals, zero_outs = [], [], [], []
    for alloc in nc.m.functions[0].allocations:
        if not isinstance(alloc, mybir.MemoryLocationSet):
            continue
        name = alloc.memorylocations[0].name
        if alloc.kind == "ExternalInput":
            if name != partition_name:
                in_names.append(name)
        elif alloc.kind == "ExternalOutput":
            out_names.append(name)
            shape = tuple(alloc.tensor_shape)
            dtype = mybir.dt.np(alloc.dtype)
            out_avals.append(jax.core.ShapedArray(shape, dtype))
            zero_outs.append(_np.zeros(shape, dtype))
    n_params = len(in_names)
    all_names = in_names + out_names
    if partition_name is not None:
        all_names = all_names + [partition_name]

    def _body(*args):
        operands = list(args)
        if partition_name is not None:
            operands.append(bass2jax.partition_id_tensor())
        for _ in range(chain):
            outs = bass2jax._bass_exec_p.bind(
                *operands,
                out_avals=tuple(out_avals),
                in_names=tuple(all_names),
                out_names=tuple(out_names),
                lowering_input_output_aliases=(),
                sim_require_finite=True,
                sim_require_nnan=True,
                nc=nc,
            )
        return tuple(outs)

    n_cores = len(in_maps)
    devices = jax.devices()[:n_cores]
    mesh = Mesh(_np.asarray(devices), ("core",))
    spec = PartitionSpec("core")
    sharded = jax.jit(
        shard_map(_body, mesh=mesh, in_specs=(spec,) * (n_params + len(out_names)),
                  out_specs=(spec,) * len(out_names), check_rep=False),
        keep_unused=True,
    )
    sharding = NamedSharding(mesh, spec)
    staged = [
        jax.device_put(
            _np.concatenate([_np.asarray(m[name]) for m in in_maps], axis=0),
            sharding)
        for name in in_names
    ] + [
        jax.device_put(
            _np.zeros((n_cores * z.shape[0], *z.shape[1:]), z.dtype), sharding)
        for z in zero_outs
    ]

    times = []
    out_arrs = None
    for _ in range(max(1, repeats)):
        t0 = time.perf_counter()
        out_arrs = jax.block_until_ready(sharded(*staged))
        times.append(time.perf_counter() - t0)

    results = [
        {
            name: _np.asarray(out_arrs[i]).reshape(n_cores, *out_avals[i].shape)[c]
            for i, name in enumerate(out_names)
        }
        for c in range(n_cores)
    ]
    return results, times


def _run(inputs, repeats=1):
    """Run on hardware. Returns (full_output, wall_times_s)."""
    in_maps = pack_inputs(inputs["x"], inputs["w"], inputs["b"])
    nc = build_nc()
    results, times = _exec(nc, in_maps, repeats=repeats)
    out = np.empty((B, OHP, OW, U), dtype=np.float32)
    for c in range(NCORES):
        # per-core out [B, OW, NO, U] -> [B, NO, OW, U]
        out[:, c * NO:(c + 1) * NO] = results[c]["out"].transpose(0, 2, 1, 3)
    return out[:, :OH], times


def kernel(x, w, b):
    out, _ = _run({"x": x, "w": w, "b": b})
    return out
